# revision 50
# baseline (speedup 1.0000x reference)
"""Multi-head attention (B=4, N=2048, C=1024, H=16) on 8 TRN2 NeuronCores.

Sharding: core c = (batch b = c//2, head-group hg = c%2), 8 heads per group.
Each core computes its head-group's attention for its batch plus the partial
output projection against the matching w_out rows; the host sums the two
partials per batch and adds the bias terms (exact: softmax rows sum to 1, so
the v-bias contributes b_v @ w_out + b_out as a constant row).

Device pipeline (per core), all matmuls bf16; both ScalarE (256 exp
activations = 285us) and the PE (~290us warm) run near their floors:
  - Heads processed in PAIRS (2g, 2g+1) inside ONE global software pipeline
    over all 4 pairs: pair p's scores occupy global steps [p*64,(p+1)*64),
    its PV trails by D=16 steps (shrinking to 8 over the last pair), so the
    score stream that feeds ScalarE never pauses at pair boundaries.
  - Per step (pair, nq-chunk j of 512, nk-tile t of 128): the two heads'
    score matmuls are K=64 row-tiles at PE positions (0,0)/(64,0) -> they
    run CONCURRENTLY in the PE array (full 128-row utilization); one
    [128,1024] psum tile holds both heads' scores and ONE ScalarE exp
    activation covers the pair.
  - PV keeps a fused ones-column (M=65) so the softmax denominator
    accumulates in psum row 64 for free.
  - Normalization: po copies emitted right after each head's last PV (fast
    psum-slot turnaround), both denominators placed at partitions 0/32 of
    one tile so a single DVE reciprocal covers both, and the PE broadcast +
    multiply are deferred 8 pv-steps so the in-order PE never parks on the
    reciprocal latency.
  - q/k/attT/x live as [128,512] chunks for fine-grained dependencies:
    attention starts while x still streams in (x rides the ScalarE HWDGE
    queue, weights the SP queue); the out-projection of token chunk jc runs
    as PE filler as soon as pair 3's chunk-jc normalize lands.
  - All projections (qkv, v, out) are deadline/quota-paced PE fillers inside
    the attention steps, plus a startup warmup burst, keeping the PE dense
    (HAM stays at K=8/8).
"""

import numpy as np

B, N, C = 4, 2048, 1024
H, Dh = 16, 64
HG = 8            # heads per core
NP = 4            # head pairs per core
P = 128
KK = C // P       # 8 contraction tiles for the projections
NT = N // P       # 16 nk tiles
NJ = N // 512     # 4 nq chunks
D = 16            # scores/exp lookahead ahead of PV (steps)

_CACHE = {}


def _build():
    import concourse.bass as bass
    import concourse.tile as tile
    from concourse import mybir, bacc
    from contextlib import ExitStack

    f32 = mybir.dt.float32
    f32r = mybir.dt.float32r
    bf16 = mybir.dt.bfloat16
    FT = mybir.ActivationFunctionType
    OP = mybir.AluOpType

    nc = bacc.Bacc("TRN2", target_bir_lowering=False, debug=False)

    import os
    KDBG = bool(os.environ.get("KDBG"))

    xT = nc.dram_tensor("xT", [C, N], bf16, kind="ExternalInput").ap()
    dbg_n = (nc.dram_tensor("dbg_n", [4, 512], mybir.dt.float32,
                            kind="ExternalOutput").ap() if KDBG else None)
    wq = nc.dram_tensor("wq", [C, 512], bf16, kind="ExternalInput").ap()
    wk = nc.dram_tensor("wk", [C, 512], bf16, kind="ExternalInput").ap()
    wv = nc.dram_tensor("wv", [C, 512], bf16, kind="ExternalInput").ap()
    bqk = nc.dram_tensor("bqk", [P, 8], f32, kind="ExternalInput").ap()
    wo = nc.dram_tensor("wo", [512, C], bf16, kind="ExternalInput").ap()
    out = nc.dram_tensor("out", [N, C], f32, kind="ExternalOutput").ap()

    with tile.TileContext(nc) as tc, ExitStack() as ctx, \
         nc.allow_low_precision(reason="bf16 attention pipeline"):
        pool = lambda name, bufs: ctx.enter_context(
            tc.tile_pool(name=name, bufs=bufs))
        qk_pool = pool("qk", 1)
        v_pool = pool("v", 1)
        attT_pool = pool("attT", 1)
        const_pool = pool("const", 1)
        x_pool = pool("x", 1)
        w_pool = pool("w", 1)
        exp_pool = pool("expst", D + 3)
        ou_pool = pool("ou", 6)
        rp_pool = pool("rp", 4)
        out_pool = pool("outst", 3)
        pscore = ctx.enter_context(
            tc.tile_pool(name="pscore", bufs=2, space="PSUM"))
        ppv = ctx.enter_context(tc.tile_pool(name="ppv", bufs=2, space="PSUM"))
        pfill = ctx.enter_context(tc.tile_pool(name="pfill", bufs=2, space="PSUM"))

        # chunked activations: [pair][j] -> [128, 512]
        q_ch = [[qk_pool.tile([P, 512], bf16, tag=f"q{g}{j}", name=f"q{g}{j}")
                 for j in range(NJ)] for g in range(NP)]
        k_ch = [[qk_pool.tile([P, 512], bf16, tag=f"k{g}{j}", name=f"k{g}{j}")
                 for j in range(NJ)] for g in range(NP)]
        attT = [[attT_pool.tile([P, 512], bf16, tag=f"a{g}{j}", name=f"a{g}{j}")
                 for j in range(NJ)] for g in range(NP)]
        vt = [v_pool.tile([P, HG * 65], bf16, tag=f"v{i}", name=f"vt{i}")
              for i in range(NT)]

        ones_f32 = const_pool.tile([33, 64], f32, tag="ones32", name="ones_f32")
        nc.vector.memset(ones_f32[:], 1.0)
        ones_t = const_pool.tile([33, 64], f32r, tag="ones", name="ones_t")
        nc.vector.tensor_copy(ones_t[:], ones_f32[:])
        biasqk_raw = const_pool.tile([P, 8], f32, tag="bqkr", name="biasqk_raw")
        nc.sync.dma_start(biasqk_raw[:], bqk)
        biasqk = const_pool.tile([P, 8], f32, tag="bqk", name="biasqk")
        nc.vector.tensor_copy(biasqk[:], biasqk_raw[:])

        # ---- input loads, ordered for earliest attention start. Early x
        # chunks ride the ScalarE HWDGE queue (idle until the first exp at
        # ~20us), weights + late x ride the SP queue: two parallel streams.
        ET = mybir.EngineType

        def load(ap, name, eng=ET.SP):
            return x_pool.tile_from(ap, name=name, forced_dma_engine=eng)

        def xload(j, eng):
            for kk in range(KK):
                xt[kk][j] = load(xT[kk * P:(kk + 1) * P, j * 512:(j + 1) * 512],
                                 f"xt{kk}_{j}", eng)

        # pair-0's k/q weight columns load as small [128,128] tiles first so
        # the first projections start ~10us earlier; the remaining 3 pairs'
        # columns follow once the critical path has launched.
        xt = [[None] * NJ for _ in range(KK)]
        wk0_t = [load(wk[kk * P:(kk + 1) * P, 0:P], f"wk0t{kk}")
                 for kk in range(KK)]
        xload(0, ET.Activation)
        wq0_t = [load(wq[kk * P:(kk + 1) * P, 0:P], f"wq0t{kk}")
                 for kk in range(KK)]
        xload(1, ET.Activation)
        xload(3, ET.SP)
        wv_t = [load(wv[kk * P:(kk + 1) * P, :], f"wvt{kk}") for kk in range(KK)]
        xload(2, ET.Activation)
        wkR_t = [load(wk[kk * P:(kk + 1) * P, P:], f"wkRt{kk}")
                 for kk in range(KK)]
        wqR_t = [load(wq[kk * P:(kk + 1) * P, P:], f"wqRt{kk}")
                 for kk in range(KK)]
        wo_t = [load(wo[kk * P:(kk + 1) * P, :], f"wot{kk}") for kk in range(4)]

        def wq_slice(g, kk):
            return wq0_t[kk][:] if g == 0 else \
                wqR_t[kk][:, (g - 1) * P:g * P]

        def wk_slice(g, kk):
            return wk0_t[kk][:] if g == 0 else \
                wkR_t[kk][:, (g - 1) * P:g * P]

        # ---- projection groups (each ~8 matmuls + 1 DVE op) -----------------
        def qproj(g, j):
            ps = pfill.tile([P, 512], f32, tag="pf", name="psa")
            for kk in range(KK):
                nc.tensor.matmul(ps[:], wq_slice(g, kk),
                                 xt[kk][j][:],
                                 start=(kk == 0), stop=(kk == KK - 1))
            nc.vector.tensor_scalar_add(q_ch[g][j][:], ps[:],
                                        biasqk[:, g:g + 1])

        def kproj(g, j):
            ps = pfill.tile([P, 512], f32, tag="pf", name="psa")
            for kk in range(KK):
                nc.tensor.matmul(ps[:], wk_slice(g, kk),
                                 xt[kk][j][:],
                                 start=(kk == 0), stop=(kk == KK - 1))
            nc.vector.tensor_scalar_add(k_ch[g][j][:], ps[:],
                                        biasqk[:, 4 + g:5 + g])

        def v_group(mg):
            ps = pfill.tile([P, 512], f32, tag="pf", name="psa")
            for kk in range(KK):
                nc.tensor.matmul(ps[:],
                                 xt[kk][mg // 4][:, (mg % 4) * P:
                                                 (mg % 4 + 1) * P],
                                 wv_t[kk][:],
                                 start=(kk == 0), stop=(kk == KK - 1))
            vg = vt[mg][:].rearrange("p (h c) -> p h c", c=65)
            nc.vector.tensor_copy(vg[:, :, 0:64],
                                  ps[:].rearrange("p (h c) -> p h c", c=64))
            nc.vector.memset(vg[:, :, 64:65], 1.0)

        def outproj(m):
            jc = m // 4
            ob = out_pool.tile([P, C], f32, tag="ob", name="ob")
            for c in range(2):
                ps = pfill.tile([P, 512], f32, tag="pf", name="psa")
                for kk in range(4):
                    nc.tensor.matmul(
                        ps[:],
                        attT[kk][jc][:, (m % 4) * P:(m % 4 + 1) * P],
                        wo_t[kk][:, c * 512:(c + 1) * 512],
                        start=(kk == 0), stop=(kk == 3))
                nc.vector.tensor_copy(ob[:, c * 512:(c + 1) * 512], ps[:])
            nc.sync.dma_start(out[m * P:(m + 1) * P, :], ob[:])

        # ---- attention: ONE global software pipeline over all 4 head
        # pairs. Pair p's scores occupy global steps [p*S, (p+1)*S); its PV
        # trails by D steps. Scores (which feed the bottleneck ScalarE exp
        # stream) therefore NEVER pause at pair boundaries.
        # fillers: list of (min_step, deadline_step, fn); deadline forces
        # emission by that step (dependency safety), min_step delays emission
        # until its producers have been emitted (out-projection snake).
        S = NJ * NT
        TOT = NP * S + 8
        es = {}
        po = {}
        pend = {}
        ocp_cur = []

        def scores_exp(g, s):
            j, t = s // NT, s % NT
            jc, off = t // 4, (t % 4) * P
            ps = pscore.tile([P, 1024], f32, tag="sc", name="psc")
            for hh in range(2):
                lo = hh * 64
                nc.tensor.matmul(ps[:, hh * 512:(hh + 1) * 512],
                                 k_ch[g][jc][lo:lo + 64, off:off + P],
                                 q_ch[g][j][lo:lo + 64, :],
                                 start=True, stop=True)
            e = exp_pool.tile([P, 1024], bf16, tag="e", name="et")
            nc.scalar.activation(e[:], ps[:], FT.Exp, scale=Dh ** -0.5)
            es[s + g * S] = e

        dd_hold = [None]

        def copy_po(hh, last=False):
            # po psum slot copy, emitted right after that head's last PV
            # matmul so the slot frees before the next chunk's first PV.
            # For the very last chunk (tail critical path) the denominator
            # row is pulled straight from psum FIRST so the reciprocal can
            # start ~2us earlier; po is never reused after it, so the extra
            # psum reader costs nothing.
            if last:
                if hh == 0:
                    dd_hold[0] = rp_pool.tile([33, 512], f32, tag="d",
                                              name="dtile")
                nc.vector.tensor_copy(dd_hold[0][32 * hh:32 * hh + 1, :],
                                      po[hh][64:65, :])
            o = ou_pool.tile([65, 512], f32, tag="o", name="otile")
            nc.vector.tensor_copy(o[:], po[hh][:])
            ocp_cur.append(o)

        def normalize_a(g, j):
            # both heads' denominators go to partitions 0/32 of one tile so
            # a SINGLE reciprocal covers both (DVE reciprocal cost scales
            # with free size only). pb broadcast + mult deferred 8 steps
            # (normalize_b) so the in-order PE never parks on the
            # reciprocal latency.
            ocp = list(ocp_cur)
            ocp_cur.clear()
            if dd_hold[0] is not None:
                dd = dd_hold[0]
                dd_hold[0] = None
            else:
                dd = rp_pool.tile([33, 512], f32, tag="d", name="dtile")
                for hh in range(2):
                    nc.vector.tensor_copy(dd[32 * hh:32 * hh + 1, :],
                                          ocp[hh][64:65, :])
            rec = rp_pool.tile([33, 512], f32r, tag="r", name="rtile")
            with nc.allow_low_precision(reason="softmax denom"):
                nc.vector.reciprocal(rec[:], dd[:])
            pend[(g, j)] = (ocp, rec)

        def normalize_b(g, j):
            ocp, rec = pend.pop((g, j))
            pbs = []
            for hh in range(2):
                lo = 32 * hh
                pb = pfill.tile([64, 512], f32, tag="pf", name="pb")
                nc.tensor.matmul(pb[:], ones_t[lo:lo + 1, :],
                                 rec[lo:lo + 1, :], start=True, stop=True)
                pbs.append(pb)
            for hh in range(2):
                nc.vector.tensor_tensor(
                    attT[g][j][hh * 64:hh * 64 + 64, :],
                    ocp[hh][0:64, :], pbs[hh][:], op=OP.mult)

        def pv(g, s):
            j, t = s // NT, s % NT
            e = es.pop(s + g * S)
            if t == 0:
                po[0] = ppv.tile([65, 512], f32, tag="po", name="po0")
                po[1] = ppv.tile([65, 512], f32, tag="po", name="po1")
            for hh in range(2):
                h = 2 * g + hh
                nc.tensor.matmul(po[hh][:], vt[t][:, h * 65:h * 65 + 65],
                                 e[:, hh * 512:(hh + 1) * 512],
                                 start=(t == 0), stop=(t == NT - 1))
                if t == NT - 1:
                    copy_po(hh, last=(g == NP - 1 and j == NJ - 1))
            if t == NT - 1:
                normalize_a(g, j)

        # ---- PE warmup: ~40 tiny back-to-back matmuls while the input DMAs
        # stream, so HAM reaches K=8/8 before the projections start
        wps = pfill.tile([64, 64], f32, tag="pf", name="wps")
        for _ in range(40):
            nc.tensor.matmul(wps[:], ones_t[0:1, :], ones_t[0:1, :],
                             start=True, stop=True)

        # ---- lead-in: minimum to start attention (k/q chunk 0-1 of pair 0);
        # everything else is deadline-forced fillers so the in-order PE never
        # parks on a DMA that hasn't landed yet.
        kproj(0, 0)
        qproj(0, 0)
        kproj(0, 1)

        fillers = [(0, 6, (lambda: kproj(0, 2))),
                   (0, 10, (lambda: kproj(0, 3)))]
        # vt[t] needed by PV(pair0, j0, t) at gs=t+D
        for t in range(NT):
            fillers.append((0, t + D - 2, (lambda t=t: v_group(t))))
        for j in range(1, NJ):
            fillers.append((0, NT * j - 2, (lambda j=j: qproj(0, j))))
        for g in range(1, NP):
            base = g * S
            # chunk jc of k feeds this pair's scores from local step 4*jc
            for j in range(NJ):
                fillers.append((0, base + 4 * j - 10,
                                (lambda g=g, j=j: kproj(g, j))))
            fillers.append((0, base - 6, (lambda g=g: qproj(g, 0))))
            for j in range(1, NJ):
                fillers.append((0, base + NT * j - 2,
                                (lambda g=g, j=j: qproj(g, j))))
        fillers = sorted(fillers, key=lambda f: f[1])
        nf = len(fillers)
        fi = 0
        # PV normally trails scores by D steps; during pair 3 it catches up
        # (lag 16 -> 4) so the post-scores PV drain at the very end shrinks.
        # The catch-up second pv() of a step never starts a new chunk (its po
        # psum slots would still be held by the pending copies).
        pv_done = 0

        def lag(gs):
            if gs < 3 * S:
                return D
            return D - max(0, min(D - 8, (gs - 3 * S) // 8))

        nb_next = 0
        ready_out = []
        for gs in range(TOT):
            pg, sl = divmod(gs, S)
            if pg < NP:
                scores_exp(pg, sl)
            npv = 0
            while (pv_done <= gs - lag(gs) and pv_done < NP * S and npv < 2
                   and not (npv == 1 and pv_done % NT == 0)):
                pv(pv_done // S, pv_done % S)
                pv_done += 1
                npv += 1
            # pb+mult for (pair, chunk) 8 pv-steps after its normalize_a;
            # once pair 3's chunk jc is normalized, its out-projection
            # becomes ready (1 unit/step keeps ACT fed)
            while nb_next < NP * NJ:
                g_, j_ = divmod(nb_next, NJ)
                if pv_done >= g_ * S + NT * j_ + NT + 8:
                    normalize_b(g_, j_)
                    if g_ == 3 and j_ < 3:
                        ready_out.extend(range(4 * j_, 4 * j_ + 4))
                    nb_next += 1
                else:
                    break
            if ready_out and (gs % NT) not in (15, 0):
                outproj(ready_out.pop(0))
            # fillers: run past-deadline ones, then fair-share quota, capped
            # at 2 units/step so ACT never starves behind a PE burst; quota
            # paused near chunk boundaries so the DVE turns po copies fast
            ran = 0
            blackout = (gs % NT) in (15, 0)
            while fi < nf and (fillers[fi][1] <= gs or
                               (fillers[fi][0] <= gs and ran < 2 and
                                not blackout and
                                fi < ((gs + 1) * nf) // (TOT - 20))):
                if fillers[fi][0] > gs:
                    break
                fillers[fi][2]()
                fi += 1
                ran += 1
        while fi < nf:
            fillers[fi][2]()
            fi += 1
        while nb_next < NP * NJ:
            g_, j_ = divmod(nb_next, NJ)
            normalize_b(g_, j_)
            nb_next += 1
        for m in ready_out:
            outproj(m)
        for m in range(12, NT):
            outproj(m)

    nc.compile()
    return nc


def _in_maps(x, w_qkv, b_qkv, w_out):
    import ml_dtypes
    bf = ml_dtypes.bfloat16
    x = np.asarray(x, np.float32)
    w_qkv = np.asarray(w_qkv, np.float32)
    b_qkv = np.asarray(b_qkv, np.float32)
    w_out = np.asarray(w_out, np.float32)
    maps = []
    for core in range(8):
        b, hg = core // 2, core % 2
        s = slice(hg * 512, hg * 512 + 512)
        maps.append({
            "xT": np.ascontiguousarray(x[b].T).astype(bf),
            "wq": np.ascontiguousarray(w_qkv[:, 0 * C:1 * C][:, s]).astype(bf),
            "wk": np.ascontiguousarray(w_qkv[:, 1 * C:2 * C][:, s]).astype(bf),
            "wv": np.ascontiguousarray(w_qkv[:, 2 * C:3 * C][:, s]).astype(bf),
            "bqk": np.ascontiguousarray(np.concatenate(
                [b_qkv[0 * C:1 * C][s], b_qkv[1 * C:2 * C][s]])
                .reshape(8, P).T),
            "wo": np.ascontiguousarray(w_out[s, :]).astype(bf),
        })
    return maps


def _gather(results, b_qkv, b_out, w_out):
    out = np.zeros((B, N, C), np.float32)
    for core in range(8):
        out[core // 2] += np.asarray(results[core]["out"], np.float32)
    # exact bias terms: softmax rows sum to 1, so +b_v contributes b_v @ w_out
    out += (np.asarray(b_qkv[2 * C:3 * C], np.float32)
            @ np.asarray(w_out, np.float32) + np.asarray(b_out, np.float32))
    return out


def run(x, w_qkv, b_qkv, w_out, b_out, trace=False):
    from concourse.bass_utils import run_bass_kernel_spmd
    if "nc" not in _CACHE:
        _CACHE["nc"] = _build()
    res = run_bass_kernel_spmd(_CACHE["nc"], _in_maps(x, w_qkv, b_qkv, w_out),
                               list(range(8)), trace=trace)
    _CACHE["last_res"] = res
    return _gather(res.results, b_qkv, b_out, w_out), res.exec_time_ns


def kernel(x, w_qkv, b_qkv, w_out, b_out):
    out, _ = run(x, w_qkv, b_qkv, w_out, b_out)
    return out


# revision 53
# speedup vs baseline: 1.0099x; 1.0099x over previous
"""Multi-head attention (B=4, N=2048, C=1024, H=16) on 8 TRN2 NeuronCores.

Sharding: core c = (batch b = c//2, head-group hg = c%2), 8 heads per group.
Each core computes its head-group's attention for its batch plus the partial
output projection against the matching w_out rows; the host sums the two
partials per batch and adds the bias terms (exact: softmax rows sum to 1, so
the v-bias contributes b_v @ w_out + b_out as a constant row).

Device pipeline (per core), all matmuls bf16; both ScalarE (256 exp
activations = 285us) and the PE (~290us warm) run near their floors:
  - Heads processed in PAIRS (2g, 2g+1) inside ONE global software pipeline
    over all 4 pairs: pair p's scores occupy global steps [p*64,(p+1)*64),
    its PV trails by D=16 steps (shrinking to 8 over the last pair), so the
    score stream that feeds ScalarE never pauses at pair boundaries.
  - Per step (pair, nq-chunk j of 512, nk-tile t of 128): the two heads'
    score matmuls are K=64 row-tiles at PE positions (0,0)/(64,0) -> they
    run CONCURRENTLY in the PE array (full 128-row utilization); one
    [128,1024] psum tile holds both heads' scores and ONE ScalarE exp
    activation covers the pair.
  - PV keeps a fused ones-column (M=65) so the softmax denominator
    accumulates in psum row 64 for free.
  - Normalization: po copies emitted right after each head's last PV (fast
    psum-slot turnaround), both denominators placed at partitions 0/32 of
    one tile so a single DVE reciprocal covers both, and the PE broadcast +
    multiply are deferred 8 pv-steps so the in-order PE never parks on the
    reciprocal latency.
  - q/k/attT/x live as [128,512] chunks for fine-grained dependencies:
    attention starts while x still streams in (x rides the ScalarE HWDGE
    queue, weights the SP queue); the out-projection of token chunk jc runs
    as PE filler as soon as pair 3's chunk-jc normalize lands.
  - All projections (qkv, v, out) are deadline/quota-paced PE fillers inside
    the attention steps, plus a startup warmup burst, keeping the PE dense
    (HAM stays at K=8/8).
"""

import numpy as np

B, N, C = 4, 2048, 1024
H, Dh = 16, 64
HG = 8            # heads per core
NP = 4            # head pairs per core
P = 128
KK = C // P       # 8 contraction tiles for the projections
NT = N // P       # 16 nk tiles
NJ = N // 512     # 4 nq chunks
D = 16            # scores/exp lookahead ahead of PV (steps)

_CACHE = {}


def _build():
    import concourse.bass as bass
    import concourse.tile as tile
    from concourse import mybir, bacc
    from contextlib import ExitStack

    f32 = mybir.dt.float32
    f32r = mybir.dt.float32r
    bf16 = mybir.dt.bfloat16
    FT = mybir.ActivationFunctionType
    OP = mybir.AluOpType

    nc = bacc.Bacc("TRN2", target_bir_lowering=False, debug=False)

    import os
    KDBG = bool(os.environ.get("KDBG"))

    xT = nc.dram_tensor("xT", [C, N], bf16, kind="ExternalInput").ap()
    dbg_n = (nc.dram_tensor("dbg_n", [4, 512], mybir.dt.float32,
                            kind="ExternalOutput").ap() if KDBG else None)
    wq = nc.dram_tensor("wq", [C, 512], bf16, kind="ExternalInput").ap()
    wk = nc.dram_tensor("wk", [C, 512], bf16, kind="ExternalInput").ap()
    wv = nc.dram_tensor("wv", [C, 512], bf16, kind="ExternalInput").ap()
    bqk = nc.dram_tensor("bqk", [P, 8], f32, kind="ExternalInput").ap()
    wo = nc.dram_tensor("wo", [512, C], bf16, kind="ExternalInput").ap()
    out = nc.dram_tensor("out", [N, C], f32, kind="ExternalOutput").ap()

    with tile.TileContext(nc) as tc, ExitStack() as ctx, \
         nc.allow_low_precision(reason="bf16 attention pipeline"):
        pool = lambda name, bufs: ctx.enter_context(
            tc.tile_pool(name=name, bufs=bufs))
        qk_pool = pool("qk", 1)
        v_pool = pool("v", 1)
        attT_pool = pool("attT", 1)
        const_pool = pool("const", 1)
        x_pool = pool("x", 1)
        w_pool = pool("w", 1)
        exp_pool = pool("expst", D + 7)
        ou_pool = pool("ou", 6)
        rp_pool = pool("rp", 3)
        out_pool = pool("outst", 2)
        pscore = ctx.enter_context(
            tc.tile_pool(name="pscore", bufs=2, space="PSUM"))
        ppv = ctx.enter_context(tc.tile_pool(name="ppv", bufs=2, space="PSUM"))
        pfill = ctx.enter_context(tc.tile_pool(name="pfill", bufs=2, space="PSUM"))

        # chunked activations: [pair][j] -> [128, 512]
        q_ch = [[qk_pool.tile([P, 512], bf16, tag=f"q{g}{j}", name=f"q{g}{j}")
                 for j in range(NJ)] for g in range(NP)]
        k_ch = [[qk_pool.tile([P, 512], bf16, tag=f"k{g}{j}", name=f"k{g}{j}")
                 for j in range(NJ)] for g in range(NP)]
        attT = [[attT_pool.tile([P, 512], bf16, tag=f"a{g}{j}", name=f"a{g}{j}")
                 for j in range(NJ)] for g in range(NP)]
        vt = [v_pool.tile([P, HG * 65], bf16, tag=f"v{i}", name=f"vt{i}")
              for i in range(NT)]

        ones_f32 = const_pool.tile([33, 64], f32, tag="ones32", name="ones_f32")
        nc.vector.memset(ones_f32[:], 1.0)
        ones_t = const_pool.tile([33, 64], f32r, tag="ones", name="ones_t")
        nc.vector.tensor_copy(ones_t[:], ones_f32[:])
        biasqk_raw = const_pool.tile([P, 8], f32, tag="bqkr", name="biasqk_raw")
        nc.sync.dma_start(biasqk_raw[:], bqk)
        biasqk = const_pool.tile([P, 8], f32, tag="bqk", name="biasqk")
        nc.vector.tensor_copy(biasqk[:], biasqk_raw[:])

        # ---- input loads, ordered for earliest attention start. Early x
        # chunks ride the ScalarE HWDGE queue (idle until the first exp at
        # ~20us), weights + late x ride the SP queue: two parallel streams.
        ET = mybir.EngineType

        def load(ap, name, eng=ET.SP):
            return x_pool.tile_from(ap, name=name, forced_dma_engine=eng)

        def xload(j, eng):
            for kk in range(KK):
                xt[kk][j] = load(xT[kk * P:(kk + 1) * P, j * 512:(j + 1) * 512],
                                 f"xt{kk}_{j}", eng)

        # pair-0's k/q weight columns load as small [128,128] tiles first so
        # the first projections start ~10us earlier; the remaining 3 pairs'
        # columns follow once the critical path has launched.
        xt = [[None] * NJ for _ in range(KK)]
        wk0_t = [load(wk[kk * P:(kk + 1) * P, 0:P], f"wk0t{kk}")
                 for kk in range(KK)]
        xload(0, ET.Activation)
        wq0_t = [load(wq[kk * P:(kk + 1) * P, 0:P], f"wq0t{kk}")
                 for kk in range(KK)]
        xload(1, ET.Activation)
        xload(3, ET.SP)
        wv_t = [load(wv[kk * P:(kk + 1) * P, :], f"wvt{kk}") for kk in range(KK)]
        xload(2, ET.Activation)
        wkR_t = [load(wk[kk * P:(kk + 1) * P, P:], f"wkRt{kk}")
                 for kk in range(KK)]
        wqR_t = [load(wq[kk * P:(kk + 1) * P, P:], f"wqRt{kk}")
                 for kk in range(KK)]
        wo_t = [load(wo[kk * P:(kk + 1) * P, :], f"wot{kk}") for kk in range(4)]

        def wq_slice(g, kk):
            return wq0_t[kk][:] if g == 0 else \
                wqR_t[kk][:, (g - 1) * P:g * P]

        def wk_slice(g, kk):
            return wk0_t[kk][:] if g == 0 else \
                wkR_t[kk][:, (g - 1) * P:g * P]

        # ---- projection groups (each ~8 matmuls + 1 DVE op) -----------------
        def qproj(g, j):
            ps = pfill.tile([P, 512], f32, tag="pf", name="psa")
            for kk in range(KK):
                nc.tensor.matmul(ps[:], wq_slice(g, kk),
                                 xt[kk][j][:],
                                 start=(kk == 0), stop=(kk == KK - 1))
            nc.vector.tensor_scalar_add(q_ch[g][j][:], ps[:],
                                        biasqk[:, g:g + 1])

        def kproj(g, j):
            ps = pfill.tile([P, 512], f32, tag="pf", name="psa")
            for kk in range(KK):
                nc.tensor.matmul(ps[:], wk_slice(g, kk),
                                 xt[kk][j][:],
                                 start=(kk == 0), stop=(kk == KK - 1))
            nc.vector.tensor_scalar_add(k_ch[g][j][:], ps[:],
                                        biasqk[:, 4 + g:5 + g])

        def v_group(mg):
            ps = pfill.tile([P, 512], f32, tag="pf", name="psa")
            for kk in range(KK):
                nc.tensor.matmul(ps[:],
                                 xt[kk][mg // 4][:, (mg % 4) * P:
                                                 (mg % 4 + 1) * P],
                                 wv_t[kk][:],
                                 start=(kk == 0), stop=(kk == KK - 1))
            vg = vt[mg][:].rearrange("p (h c) -> p h c", c=65)
            nc.vector.tensor_copy(vg[:, :, 0:64],
                                  ps[:].rearrange("p (h c) -> p h c", c=64))
            nc.vector.memset(vg[:, :, 64:65], 1.0)

        def outproj(m):
            jc = m // 4
            ob = out_pool.tile([P, C], f32, tag="ob", name="ob")
            for c in range(2):
                ps = pfill.tile([P, 512], f32, tag="pf", name="psa")
                for kk in range(4):
                    nc.tensor.matmul(
                        ps[:],
                        attT[kk][jc][:, (m % 4) * P:(m % 4 + 1) * P],
                        wo_t[kk][:, c * 512:(c + 1) * 512],
                        start=(kk == 0), stop=(kk == 3))
                nc.vector.tensor_copy(ob[:, c * 512:(c + 1) * 512], ps[:])
            nc.sync.dma_start(out[m * P:(m + 1) * P, :], ob[:])

        # ---- attention: ONE global software pipeline over all 4 head
        # pairs. Pair p's scores occupy global steps [p*S, (p+1)*S); its PV
        # trails by D steps. Scores (which feed the bottleneck ScalarE exp
        # stream) therefore NEVER pause at pair boundaries.
        # fillers: list of (min_step, deadline_step, fn); deadline forces
        # emission by that step (dependency safety), min_step delays emission
        # until its producers have been emitted (out-projection snake).
        S = NJ * NT
        TOT = NP * S + 8
        es = {}
        po = {}
        pend = {}
        ocp_cur = []

        def scores_exp(g, s):
            j, t = s // NT, s % NT
            jc, off = t // 4, (t % 4) * P
            ps = pscore.tile([P, 1024], f32, tag="sc", name="psc")
            for hh in range(2):
                lo = hh * 64
                nc.tensor.matmul(ps[:, hh * 512:(hh + 1) * 512],
                                 k_ch[g][jc][lo:lo + 64, off:off + P],
                                 q_ch[g][j][lo:lo + 64, :],
                                 start=True, stop=True)
            e = exp_pool.tile([P, 1024], bf16, tag="e", name="et")
            nc.scalar.activation(e[:], ps[:], FT.Exp, scale=Dh ** -0.5)
            es[s + g * S] = e

        dd_hold = [None]

        def copy_po(hh, last=False):
            # po psum slot copy, emitted right after that head's last PV
            # matmul so the slot frees before the next chunk's first PV.
            # For the very last chunk (tail critical path) the denominator
            # row is pulled straight from psum FIRST so the reciprocal can
            # start ~2us earlier; po is never reused after it, so the extra
            # psum reader costs nothing.
            if last:
                if hh == 0:
                    dd_hold[0] = rp_pool.tile([33, 512], f32, tag="d",
                                              name="dtile")
                nc.vector.tensor_copy(dd_hold[0][32 * hh:32 * hh + 1, :],
                                      po[hh][64:65, :])
            o = ou_pool.tile([65, 512], f32, tag="o", name="otile")
            nc.vector.tensor_copy(o[:], po[hh][:])
            ocp_cur.append(o)

        def normalize_a(g, j):
            # both heads' denominators go to partitions 0/32 of one tile so
            # a SINGLE reciprocal covers both (DVE reciprocal cost scales
            # with free size only). pb broadcast + mult deferred 8 steps
            # (normalize_b) so the in-order PE never parks on the
            # reciprocal latency.
            ocp = list(ocp_cur)
            ocp_cur.clear()
            if dd_hold[0] is not None:
                dd = dd_hold[0]
                dd_hold[0] = None
            else:
                dd = rp_pool.tile([33, 512], f32, tag="d", name="dtile")
                for hh in range(2):
                    nc.vector.tensor_copy(dd[32 * hh:32 * hh + 1, :],
                                          ocp[hh][64:65, :])
            rec = rp_pool.tile([33, 512], f32r, tag="r", name="rtile")
            with nc.allow_low_precision(reason="softmax denom"):
                nc.vector.reciprocal(rec[:], dd[:])
            pend[(g, j)] = (ocp, rec)

        def normalize_b(g, j):
            ocp, rec = pend.pop((g, j))
            pbs = []
            for hh in range(2):
                lo = 32 * hh
                pb = pfill.tile([64, 512], f32, tag="pf", name="pb")
                nc.tensor.matmul(pb[:], ones_t[lo:lo + 1, :],
                                 rec[lo:lo + 1, :], start=True, stop=True)
                pbs.append(pb)
            for hh in range(2):
                nc.vector.tensor_tensor(
                    attT[g][j][hh * 64:hh * 64 + 64, :],
                    ocp[hh][0:64, :], pbs[hh][:], op=OP.mult)

        def pv(g, s):
            j, t = s // NT, s % NT
            e = es.pop(s + g * S)
            if t == 0:
                po[0] = ppv.tile([65, 512], f32, tag="po", name="po0")
                po[1] = ppv.tile([65, 512], f32, tag="po", name="po1")
            for hh in range(2):
                h = 2 * g + hh
                nc.tensor.matmul(po[hh][:], vt[t][:, h * 65:h * 65 + 65],
                                 e[:, hh * 512:(hh + 1) * 512],
                                 start=(t == 0), stop=(t == NT - 1))
                if t == NT - 1:
                    copy_po(hh, last=(g == NP - 1 and j == NJ - 1))
            if t == NT - 1:
                normalize_a(g, j)

        # ---- PE warmup: ~40 tiny back-to-back matmuls while the input DMAs
        # stream, so HAM reaches K=8/8 before the projections start
        wps = pfill.tile([64, 64], f32, tag="pf", name="wps")
        for _ in range(40):
            nc.tensor.matmul(wps[:], ones_t[0:1, :], ones_t[0:1, :],
                             start=True, stop=True)

        # ---- lead-in: minimum to start attention (k/q chunk 0-1 of pair 0);
        # everything else is deadline-forced fillers so the in-order PE never
        # parks on a DMA that hasn't landed yet.
        kproj(0, 0)
        qproj(0, 0)
        kproj(0, 1)

        fillers = [(0, 6, (lambda: kproj(0, 2))),
                   (0, 10, (lambda: kproj(0, 3)))]
        # vt[t] needed by PV(pair0, j0, t) at gs ~ t + 24 (initial lag)
        for t in range(NT):
            fillers.append((0, t + D + 1, (lambda t=t: v_group(t))))
        for j in range(1, NJ):
            fillers.append((0, NT * j - 2, (lambda j=j: qproj(0, j))))
        for g in range(1, NP):
            base = g * S
            # chunk jc of k feeds this pair's scores from local step 4*jc
            for j in range(NJ):
                fillers.append((0, base + 4 * j - 10,
                                (lambda g=g, j=j: kproj(g, j))))
            fillers.append((0, base - 6, (lambda g=g: qproj(g, 0))))
            for j in range(1, NJ):
                fillers.append((0, base + NT * j - 2,
                                (lambda g=g, j=j: qproj(g, j))))
        fillers = sorted(fillers, key=lambda f: f[1])
        nf = len(fillers)
        fi = 0
        # PV normally trails scores by D steps; during pair 3 it catches up
        # (lag 16 -> 4) so the post-scores PV drain at the very end shrinks.
        # The catch-up second pv() of a step never starts a new chunk (its po
        # psum slots would still be held by the pending copies).
        pv_done = 0

        def lag(gs):
            # 24 easing to 16 across pair 0 (spreads the deadline-forced
            # v_groups over ~38 steps instead of 16, halving pair-0's ACT
            # starvation), constant 16 mid-stream, easing to 8 over pair 3
            # (shrinks the post-scores PV drain).
            if gs < S:
                return (D + 4) - min(4, gs // 16)
            if gs < 3 * S:
                return D
            return D - max(0, min(D - 8, (gs - 3 * S) // 8))

        nb_next = 0
        ready_out = []
        for gs in range(TOT):
            pg, sl = divmod(gs, S)
            if pg < NP:
                scores_exp(pg, sl)
            npv = 0
            while (pv_done <= gs - lag(gs) and pv_done < NP * S and npv < 2
                   and not (npv == 1 and pv_done % NT == 0)):
                pv(pv_done // S, pv_done % S)
                pv_done += 1
                npv += 1
            # pb+mult for (pair, chunk) 8 pv-steps after its normalize_a;
            # once pair 3's chunk jc is normalized, its out-projection
            # becomes ready (1 unit/step keeps ACT fed)
            while nb_next < NP * NJ:
                g_, j_ = divmod(nb_next, NJ)
                if pv_done >= g_ * S + NT * j_ + NT + 8:
                    normalize_b(g_, j_)
                    if g_ == 3 and j_ < 3:
                        ready_out.extend(range(4 * j_, 4 * j_ + 4))
                    nb_next += 1
                else:
                    break
            if ready_out and (gs % NT) not in (15, 0):
                outproj(ready_out.pop(0))
            # fillers: run past-deadline ones, then fair-share quota, capped
            # at 2 units/step so ACT never starves behind a PE burst; quota
            # paused near chunk boundaries so the DVE turns po copies fast
            ran = 0
            blackout = (gs % NT) in (15, 0)
            while fi < nf and (fillers[fi][1] <= gs or
                               (fillers[fi][0] <= gs and ran < 2 and
                                not blackout and
                                fi < ((gs + 1) * nf) // (TOT - 20))):
                if fillers[fi][0] > gs:
                    break
                fillers[fi][2]()
                fi += 1
                ran += 1
        while fi < nf:
            fillers[fi][2]()
            fi += 1
        while nb_next < NP * NJ:
            g_, j_ = divmod(nb_next, NJ)
            normalize_b(g_, j_)
            nb_next += 1
        for m in ready_out:
            outproj(m)
        for m in range(12, NT):
            outproj(m)

    nc.compile()
    return nc


def _in_maps(x, w_qkv, b_qkv, w_out):
    import ml_dtypes
    bf = ml_dtypes.bfloat16
    x = np.asarray(x, np.float32)
    w_qkv = np.asarray(w_qkv, np.float32)
    b_qkv = np.asarray(b_qkv, np.float32)
    w_out = np.asarray(w_out, np.float32)
    maps = []
    for core in range(8):
        b, hg = core // 2, core % 2
        s = slice(hg * 512, hg * 512 + 512)
        maps.append({
            "xT": np.ascontiguousarray(x[b].T).astype(bf),
            "wq": np.ascontiguousarray(w_qkv[:, 0 * C:1 * C][:, s]).astype(bf),
            "wk": np.ascontiguousarray(w_qkv[:, 1 * C:2 * C][:, s]).astype(bf),
            "wv": np.ascontiguousarray(w_qkv[:, 2 * C:3 * C][:, s]).astype(bf),
            "bqk": np.ascontiguousarray(np.concatenate(
                [b_qkv[0 * C:1 * C][s], b_qkv[1 * C:2 * C][s]])
                .reshape(8, P).T),
            "wo": np.ascontiguousarray(w_out[s, :]).astype(bf),
        })
    return maps


def _gather(results, b_qkv, b_out, w_out):
    out = np.zeros((B, N, C), np.float32)
    for core in range(8):
        out[core // 2] += np.asarray(results[core]["out"], np.float32)
    # exact bias terms: softmax rows sum to 1, so +b_v contributes b_v @ w_out
    out += (np.asarray(b_qkv[2 * C:3 * C], np.float32)
            @ np.asarray(w_out, np.float32) + np.asarray(b_out, np.float32))
    return out


def run(x, w_qkv, b_qkv, w_out, b_out, trace=False):
    from concourse.bass_utils import run_bass_kernel_spmd
    if "nc" not in _CACHE:
        _CACHE["nc"] = _build()
    res = run_bass_kernel_spmd(_CACHE["nc"], _in_maps(x, w_qkv, b_qkv, w_out),
                               list(range(8)), trace=trace)
    _CACHE["last_res"] = res
    return _gather(res.results, b_qkv, b_out, w_out), res.exec_time_ns


def kernel(x, w_qkv, b_qkv, w_out, b_out):
    out, _ = run(x, w_qkv, b_qkv, w_out, b_out)
    return out


# revision 54
# speedup vs baseline: 1.0161x; 1.0062x over previous
"""Multi-head attention (B=4, N=2048, C=1024, H=16) on 8 TRN2 NeuronCores.

Sharding: core c = (batch b = c//2, head-group hg = c%2), 8 heads per group.
Each core computes its head-group's attention for its batch plus the partial
output projection against the matching w_out rows; the host sums the two
partials per batch and adds the bias terms (exact: softmax rows sum to 1, so
the v-bias contributes b_v @ w_out + b_out as a constant row).

Device pipeline (per core), all matmuls bf16; both ScalarE (256 exp
activations = 285us) and the PE (~290us warm) run near their floors:
  - Heads processed in PAIRS (2g, 2g+1) inside ONE global software pipeline
    over all 4 pairs: pair p's scores occupy global steps [p*64,(p+1)*64),
    its PV trails by D=16 steps (shrinking to 8 over the last pair), so the
    score stream that feeds ScalarE never pauses at pair boundaries.
  - Per step (pair, nq-chunk j of 512, nk-tile t of 128): the two heads'
    score matmuls are K=64 row-tiles at PE positions (0,0)/(64,0) -> they
    run CONCURRENTLY in the PE array (full 128-row utilization); one
    [128,1024] psum tile holds both heads' scores and ONE ScalarE exp
    activation covers the pair.
  - PV keeps a fused ones-column (M=65) so the softmax denominator
    accumulates in psum row 64 for free.
  - Normalization: po copies emitted right after each head's last PV (fast
    psum-slot turnaround), both denominators placed at partitions 0/32 of
    one tile so a single DVE reciprocal covers both, and the PE broadcast +
    multiply are deferred 8 pv-steps so the in-order PE never parks on the
    reciprocal latency.
  - q/k/attT/x live as [128,512] chunks for fine-grained dependencies:
    attention starts while x still streams in (x rides the ScalarE HWDGE
    queue, weights the SP queue); the out-projection of token chunk jc runs
    as PE filler as soon as pair 3's chunk-jc normalize lands.
  - All projections (qkv, v, out) are deadline/quota-paced PE fillers inside
    the attention steps, plus a startup warmup burst, keeping the PE dense
    (HAM stays at K=8/8).
"""

import numpy as np

B, N, C = 4, 2048, 1024
H, Dh = 16, 64
HG = 8            # heads per core
NP = 4            # head pairs per core
P = 128
KK = C // P       # 8 contraction tiles for the projections
NT = N // P       # 16 nk tiles
NJ = N // 512     # 4 nq chunks
D = 16            # scores/exp lookahead ahead of PV (steps)

_CACHE = {}


def _build():
    import concourse.bass as bass
    import concourse.tile as tile
    from concourse import mybir, bacc
    from contextlib import ExitStack

    f32 = mybir.dt.float32
    f32r = mybir.dt.float32r
    bf16 = mybir.dt.bfloat16
    FT = mybir.ActivationFunctionType
    OP = mybir.AluOpType

    nc = bacc.Bacc("TRN2", target_bir_lowering=False, debug=False)

    import os
    KDBG = bool(os.environ.get("KDBG"))

    xT = nc.dram_tensor("xT", [C, N], bf16, kind="ExternalInput").ap()
    dbg_n = (nc.dram_tensor("dbg_n", [4, 512], mybir.dt.float32,
                            kind="ExternalOutput").ap() if KDBG else None)
    wq = nc.dram_tensor("wq", [C, 512], bf16, kind="ExternalInput").ap()
    wk = nc.dram_tensor("wk", [C, 512], bf16, kind="ExternalInput").ap()
    wv = nc.dram_tensor("wv", [C, 512], bf16, kind="ExternalInput").ap()
    bqk = nc.dram_tensor("bqk", [P, 8], f32, kind="ExternalInput").ap()
    wo = nc.dram_tensor("wo", [512, C], bf16, kind="ExternalInput").ap()
    out = nc.dram_tensor("out", [N, C], f32, kind="ExternalOutput").ap()

    with tile.TileContext(nc) as tc, ExitStack() as ctx, \
         nc.allow_low_precision(reason="bf16 attention pipeline"):
        pool = lambda name, bufs: ctx.enter_context(
            tc.tile_pool(name=name, bufs=bufs))
        qk_pool = pool("qk", 1)
        v_pool = pool("v", 1)
        attT_pool = pool("attT", 1)
        const_pool = pool("const", 1)
        x_pool = pool("x", 1)
        w_pool = pool("w", 1)
        exp_pool = pool("expst", D + 3)
        ou_pool = pool("ou", 6)
        rp_pool = pool("rp", 4)
        out_pool = pool("outst", 3)
        pscore = ctx.enter_context(
            tc.tile_pool(name="pscore", bufs=2, space="PSUM"))
        ppv = ctx.enter_context(tc.tile_pool(name="ppv", bufs=2, space="PSUM"))
        pfill = ctx.enter_context(tc.tile_pool(name="pfill", bufs=2, space="PSUM"))

        # chunked activations: [pair][j] -> [128, 512]
        q_ch = [[qk_pool.tile([P, 512], bf16, tag=f"q{g}{j}", name=f"q{g}{j}")
                 for j in range(NJ)] for g in range(NP)]
        k_ch = [[qk_pool.tile([P, 512], bf16, tag=f"k{g}{j}", name=f"k{g}{j}")
                 for j in range(NJ)] for g in range(NP)]
        attT = [[attT_pool.tile([P, 512], bf16, tag=f"a{g}{j}", name=f"a{g}{j}")
                 for j in range(NJ)] for g in range(NP)]
        vt = [v_pool.tile([P, HG * 65], bf16, tag=f"v{i}", name=f"vt{i}")
              for i in range(NT)]

        ones_f32 = const_pool.tile([33, 64], f32, tag="ones32", name="ones_f32")
        nc.vector.memset(ones_f32[:], 1.0)
        ones_t = const_pool.tile([33, 64], f32r, tag="ones", name="ones_t")
        nc.vector.tensor_copy(ones_t[:], ones_f32[:])
        biasqk_raw = const_pool.tile([P, 8], f32, tag="bqkr", name="biasqk_raw")
        nc.sync.dma_start(biasqk_raw[:], bqk)
        biasqk = const_pool.tile([P, 8], f32, tag="bqk", name="biasqk")
        nc.vector.tensor_copy(biasqk[:], biasqk_raw[:])

        # ---- input loads, ordered for earliest attention start. Early x
        # chunks ride the ScalarE HWDGE queue (idle until the first exp at
        # ~20us), weights + late x ride the SP queue: two parallel streams.
        ET = mybir.EngineType

        def load(ap, name, eng=ET.SP):
            return x_pool.tile_from(ap, name=name, forced_dma_engine=eng)

        def xload(j, eng):
            for kk in range(KK):
                xt[kk][j] = load(xT[kk * P:(kk + 1) * P, j * 512:(j + 1) * 512],
                                 f"xt{kk}_{j}", eng)

        # pair-0's k/q weight columns load as small [128,128] tiles first so
        # the first projections start ~10us earlier; the remaining 3 pairs'
        # columns follow once the critical path has launched.
        xt = [[None] * NJ for _ in range(KK)]
        wk0_t = [load(wk[kk * P:(kk + 1) * P, 0:P], f"wk0t{kk}")
                 for kk in range(KK)]
        xload(0, ET.Activation)
        wq0_t = [load(wq[kk * P:(kk + 1) * P, 0:P], f"wq0t{kk}")
                 for kk in range(KK)]
        xload(1, ET.Activation)
        xload(3, ET.SP)
        wv_t = [load(wv[kk * P:(kk + 1) * P, :], f"wvt{kk}") for kk in range(KK)]
        xload(2, ET.Activation)
        wkR_t = [load(wk[kk * P:(kk + 1) * P, P:], f"wkRt{kk}")
                 for kk in range(KK)]
        wqR_t = [load(wq[kk * P:(kk + 1) * P, P:], f"wqRt{kk}")
                 for kk in range(KK)]
        wo_t = [load(wo[kk * P:(kk + 1) * P, :], f"wot{kk}") for kk in range(4)]

        def wq_slice(g, kk):
            return wq0_t[kk][:] if g == 0 else \
                wqR_t[kk][:, (g - 1) * P:g * P]

        def wk_slice(g, kk):
            return wk0_t[kk][:] if g == 0 else \
                wkR_t[kk][:, (g - 1) * P:g * P]

        # ---- projection groups (each ~8 matmuls + 1 DVE op) -----------------
        def qproj(g, j):
            ps = pfill.tile([P, 512], f32, tag="pf", name="psa")
            for kk in range(KK):
                nc.tensor.matmul(ps[:], wq_slice(g, kk),
                                 xt[kk][j][:],
                                 start=(kk == 0), stop=(kk == KK - 1))
            nc.vector.tensor_scalar_add(q_ch[g][j][:], ps[:],
                                        biasqk[:, g:g + 1])

        def kproj(g, j):
            ps = pfill.tile([P, 512], f32, tag="pf", name="psa")
            for kk in range(KK):
                nc.tensor.matmul(ps[:], wk_slice(g, kk),
                                 xt[kk][j][:],
                                 start=(kk == 0), stop=(kk == KK - 1))
            nc.vector.tensor_scalar_add(k_ch[g][j][:], ps[:],
                                        biasqk[:, 4 + g:5 + g])

        def v_group(mg):
            ps = pfill.tile([P, 512], f32, tag="pf", name="psa")
            for kk in range(KK):
                nc.tensor.matmul(ps[:],
                                 xt[kk][mg // 4][:, (mg % 4) * P:
                                                 (mg % 4 + 1) * P],
                                 wv_t[kk][:],
                                 start=(kk == 0), stop=(kk == KK - 1))
            vg = vt[mg][:].rearrange("p (h c) -> p h c", c=65)
            nc.vector.tensor_copy(vg[:, :, 0:64],
                                  ps[:].rearrange("p (h c) -> p h c", c=64))
            nc.vector.memset(vg[:, :, 64:65], 1.0)

        def outproj(m):
            jc = m // 4
            ob = out_pool.tile([P, C], f32, tag="ob", name="ob")
            for c in range(2):
                ps = pfill.tile([P, 512], f32, tag="pf", name="psa")
                for kk in range(4):
                    nc.tensor.matmul(
                        ps[:],
                        attT[kk][jc][:, (m % 4) * P:(m % 4 + 1) * P],
                        wo_t[kk][:, c * 512:(c + 1) * 512],
                        start=(kk == 0), stop=(kk == 3))
                nc.vector.tensor_copy(ob[:, c * 512:(c + 1) * 512], ps[:])
            nc.sync.dma_start(out[m * P:(m + 1) * P, :], ob[:])

        # ---- attention: ONE global software pipeline over all 4 head
        # pairs. Pair p's scores occupy global steps [p*S, (p+1)*S); its PV
        # trails by D steps. Scores (which feed the bottleneck ScalarE exp
        # stream) therefore NEVER pause at pair boundaries.
        # fillers: list of (min_step, deadline_step, fn); deadline forces
        # emission by that step (dependency safety), min_step delays emission
        # until its producers have been emitted (out-projection snake).
        S = NJ * NT
        TOT = NP * S + 8
        es = {}
        po = {}
        pend = {}
        ocp_cur = []

        def scores_exp(g, s):
            j, t = s // NT, s % NT
            jc, off = t // 4, (t % 4) * P
            ps = pscore.tile([P, 1024], f32, tag="sc", name="psc")
            for hh in range(2):
                lo = hh * 64
                nc.tensor.matmul(ps[:, hh * 512:(hh + 1) * 512],
                                 k_ch[g][jc][lo:lo + 64, off:off + P],
                                 q_ch[g][j][lo:lo + 64, :],
                                 start=True, stop=True)
            e = exp_pool.tile([P, 1024], bf16, tag="e", name="et")
            nc.scalar.activation(e[:], ps[:], FT.Exp, scale=Dh ** -0.5)
            es[s + g * S] = e

        dd_hold = [None]

        def copy_po(hh, last=False):
            # po psum slot copy, emitted right after that head's last PV
            # matmul so the slot frees before the next chunk's first PV.
            # For the very last chunk (tail critical path) the denominator
            # row is pulled straight from psum FIRST so the reciprocal can
            # start ~2us earlier; po is never reused after it, so the extra
            # psum reader costs nothing.
            if last:
                if hh == 0:
                    dd_hold[0] = rp_pool.tile([33, 512], f32, tag="d",
                                              name="dtile")
                nc.vector.tensor_copy(dd_hold[0][32 * hh:32 * hh + 1, :],
                                      po[hh][64:65, :])
            o = ou_pool.tile([65, 512], f32, tag="o", name="otile")
            nc.vector.tensor_copy(o[:], po[hh][:])
            ocp_cur.append(o)

        def normalize_a(g, j):
            # both heads' denominators go to partitions 0/32 of one tile so
            # a SINGLE reciprocal covers both (DVE reciprocal cost scales
            # with free size only). pb broadcast + mult deferred 8 steps
            # (normalize_b) so the in-order PE never parks on the
            # reciprocal latency.
            ocp = list(ocp_cur)
            ocp_cur.clear()
            if dd_hold[0] is not None:
                dd = dd_hold[0]
                dd_hold[0] = None
            else:
                dd = rp_pool.tile([33, 512], f32, tag="d", name="dtile")
                for hh in range(2):
                    nc.vector.tensor_copy(dd[32 * hh:32 * hh + 1, :],
                                          ocp[hh][64:65, :])
            rec = rp_pool.tile([33, 512], f32r, tag="r", name="rtile")
            with nc.allow_low_precision(reason="softmax denom"):
                nc.vector.reciprocal(rec[:], dd[:])
            pend[(g, j)] = (ocp, rec)

        def normalize_b(g, j):
            ocp, rec = pend.pop((g, j))
            pbs = []
            for hh in range(2):
                lo = 32 * hh
                pb = pfill.tile([64, 512], f32, tag="pf", name="pb")
                nc.tensor.matmul(pb[:], ones_t[lo:lo + 1, :],
                                 rec[lo:lo + 1, :], start=True, stop=True)
                pbs.append(pb)
            for hh in range(2):
                nc.vector.tensor_tensor(
                    attT[g][j][hh * 64:hh * 64 + 64, :],
                    ocp[hh][0:64, :], pbs[hh][:], op=OP.mult)

        def pv(g, s):
            j, t = s // NT, s % NT
            e = es.pop(s + g * S)
            if t == 0:
                po[0] = ppv.tile([65, 512], f32, tag="po", name="po0")
                po[1] = ppv.tile([65, 512], f32, tag="po", name="po1")
            for hh in range(2):
                h = 2 * g + hh
                nc.tensor.matmul(po[hh][:], vt[t][:, h * 65:h * 65 + 65],
                                 e[:, hh * 512:(hh + 1) * 512],
                                 start=(t == 0), stop=(t == NT - 1))
                if t == NT - 1:
                    copy_po(hh, last=(g == NP - 1 and j == NJ - 1))
            if t == NT - 1:
                normalize_a(g, j)

        # ---- PE warmup: ~40 tiny back-to-back matmuls while the input DMAs
        # stream, so HAM reaches K=8/8 before the projections start
        wps = pfill.tile([64, 64], f32, tag="pf", name="wps")
        for _ in range(40):
            nc.tensor.matmul(wps[:], ones_t[0:1, :], ones_t[0:1, :],
                             start=True, stop=True)

        # ---- lead-in: minimum to start attention (k/q chunk 0-1 of pair 0);
        # everything else is deadline-forced fillers so the in-order PE never
        # parks on a DMA that hasn't landed yet.
        kproj(0, 0)
        qproj(0, 0)
        kproj(0, 1)

        fillers = [(0, 6, (lambda: kproj(0, 2))),
                   (0, 10, (lambda: kproj(0, 3)))]
        # vt[t] needed by PV(pair0, j0, t) at gs=t+D
        for t in range(NT):
            fillers.append((0, t + D - 2, (lambda t=t: v_group(t))))
        for j in range(1, NJ):
            fillers.append((0, NT * j - 2, (lambda j=j: qproj(0, j))))
        for g in range(1, NP):
            base = g * S
            # chunk jc of k feeds this pair's scores from local step 4*jc
            for j in range(NJ):
                fillers.append((0, base + 4 * j - 10,
                                (lambda g=g, j=j: kproj(g, j))))
            fillers.append((0, base - 6, (lambda g=g: qproj(g, 0))))
            for j in range(1, NJ):
                fillers.append((0, base + NT * j - 2,
                                (lambda g=g, j=j: qproj(g, j))))
        fillers = sorted(fillers, key=lambda f: f[1])
        nf = len(fillers)
        fi = 0
        # PV normally trails scores by D steps; during pair 3 it catches up
        # (lag 16 -> 4) so the post-scores PV drain at the very end shrinks.
        # The catch-up second pv() of a step never starts a new chunk (its po
        # psum slots would still be held by the pending copies).
        pv_done = 0

        def lag(gs):
            if gs < 3 * S:
                return D
            return D - max(0, min(D - 8, (gs - 3 * S) // 8))

        nb_next = 0
        ready_out = []
        for gs in range(TOT):
            pg, sl = divmod(gs, S)
            if pg < NP:
                scores_exp(pg, sl)
            npv = 0
            while (pv_done <= gs - lag(gs) and pv_done < NP * S and npv < 2
                   and not (npv == 1 and pv_done % NT == 0)):
                pv(pv_done // S, pv_done % S)
                pv_done += 1
                npv += 1
            # pb+mult for (pair, chunk) 8 pv-steps after its normalize_a;
            # once pair 3's chunk jc is normalized, its out-projection
            # becomes ready (1 unit/step keeps ACT fed)
            while nb_next < NP * NJ:
                g_, j_ = divmod(nb_next, NJ)
                if pv_done >= g_ * S + NT * j_ + NT + 8:
                    normalize_b(g_, j_)
                    if g_ == 3 and j_ < 3:
                        ready_out.extend(range(4 * j_, 4 * j_ + 4))
                    nb_next += 1
                else:
                    break
            if ready_out and (gs % NT) not in (15, 0):
                outproj(ready_out.pop(0))
            # fillers: run past-deadline ones, then fair-share quota, capped
            # at 2 units/step so ACT never starves behind a PE burst; quota
            # paused near chunk boundaries so the DVE turns po copies fast
            ran = 0
            blackout = (gs % NT) in (15, 0)
            while fi < nf and (fillers[fi][1] <= gs or
                               (fillers[fi][0] <= gs and ran < 2 and
                                not blackout and
                                fi < ((gs + 1) * nf) // (TOT - 20))):
                if fillers[fi][0] > gs:
                    break
                fillers[fi][2]()
                fi += 1
                ran += 1
        while fi < nf:
            fillers[fi][2]()
            fi += 1
        while nb_next < NP * NJ:
            g_, j_ = divmod(nb_next, NJ)
            normalize_b(g_, j_)
            nb_next += 1
        for m in ready_out:
            outproj(m)
        for m in range(12, NT):
            outproj(m)

    nc.compile()
    return nc


def _in_maps(x, w_qkv, b_qkv, w_out):
    import ml_dtypes
    bf = ml_dtypes.bfloat16
    x = np.asarray(x, np.float32)
    w_qkv = np.asarray(w_qkv, np.float32)
    b_qkv = np.asarray(b_qkv, np.float32)
    w_out = np.asarray(w_out, np.float32)
    maps = []
    for core in range(8):
        b, hg = core // 2, core % 2
        s = slice(hg * 512, hg * 512 + 512)
        maps.append({
            "xT": np.ascontiguousarray(x[b].T).astype(bf),
            "wq": np.ascontiguousarray(w_qkv[:, 0 * C:1 * C][:, s]).astype(bf),
            "wk": np.ascontiguousarray(w_qkv[:, 1 * C:2 * C][:, s]).astype(bf),
            "wv": np.ascontiguousarray(w_qkv[:, 2 * C:3 * C][:, s]).astype(bf),
            "bqk": np.ascontiguousarray(np.concatenate(
                [b_qkv[0 * C:1 * C][s], b_qkv[1 * C:2 * C][s]])
                .reshape(8, P).T),
            "wo": np.ascontiguousarray(w_out[s, :]).astype(bf),
        })
    return maps


def _gather(results, b_qkv, b_out, w_out):
    out = np.zeros((B, N, C), np.float32)
    for core in range(8):
        out[core // 2] += np.asarray(results[core]["out"], np.float32)
    # exact bias terms: softmax rows sum to 1, so +b_v contributes b_v @ w_out
    out += (np.asarray(b_qkv[2 * C:3 * C], np.float32)
            @ np.asarray(w_out, np.float32) + np.asarray(b_out, np.float32))
    return out


def run(x, w_qkv, b_qkv, w_out, b_out, trace=False):
    from concourse.bass_utils import run_bass_kernel_spmd
    if "nc" not in _CACHE:
        _CACHE["nc"] = _build()
    res = run_bass_kernel_spmd(_CACHE["nc"], _in_maps(x, w_qkv, b_qkv, w_out),
                               list(range(8)), trace=trace)
    _CACHE["last_res"] = res
    return _gather(res.results, b_qkv, b_out, w_out), res.exec_time_ns


def kernel(x, w_qkv, b_qkv, w_out, b_out):
    out, _ = run(x, w_qkv, b_qkv, w_out, b_out)
    return out
